# revision 23
# baseline (speedup 1.0000x reference)
"""Trainium2 Bass kernel for nn_Encoder (MoE routing encoder).

The encoder's per-token pre-expert state is a pure table lookup: view 0
depends only on the vocab id (src) and views 1/2 only on the quantized
fractional-encoding index, so the embedding/positional lookups fold with the
per-view projection and router weights into [VOCAB,64] / [RES,64] tables
(host, float64).  Tokens sharing a table row are DEDUPLICATED into "units":
the router distances, top-4 gates and the expert MLP outputs are identical
for every token mapping to the same unit, so the device only evaluates each
unique (view, table-row) once (~2.9x fewer expert-MLP evaluations).

Host: routes each unit (Laplace top-4 + softmax gates, float64), groups the
(unit, expert) assignments by expert, splits oversized experts into even
pieces, and places pieces on the 8 cores by rank-LPT: pieces sorted by
width, consecutive blocks of 8 go one to each core (largest to the
least-loaded), so every core shares one per-rank width profile (the
all-to-all token dispatch of the sharding hint, done during sharding).
Slots are packed densely (no 128-alignment); per-core packs are fp16.

Device (one SPMD module, 8 cores): per piece, y1 = gelu(x @ W1[e]) with
both W1 halves sharing one [128, *] weight tensor (X is duplicated into
partitions 64-127 so the half-1 matmul sees matching base partitions), y2
accumulated in PSUM as [64, slots] via W2-stationary matmuls, gelu batched
over rank groups, outputs copied fp16 and streamed back by DMA.  Bulk
weights are triggered from the Pool engine (SWDGE) so the single HWDGE
trigger device stays free for the latency-critical X and output DMAs.

Unsharding (host): gate-weighted 4-way gather-sum per unit, token gather
per view, plus the gate-weighted b2 term and the hmask.
"""

import contextlib

import numpy as np

import concourse.bacc as bacc
import concourse.mybir as mybir
import concourse.tile as tile

F32 = mybir.dt.float32
F16 = mybir.dt.float16
AF = mybir.ActivationFunctionType

B, T, D, E, D4 = 128, 16, 64, 64, 256
RES, FEAT, VOCAB = 5000, 200, 119
N_CORES = 8
NV = 3                                # views
K = 4                                 # top-k experts
NTOK = B * T

_CACHE = {}


def kernel(**inputs):
    from concourse.bass_utils import run_bass_kernel_spmd

    rt = _route(inputs)
    key = ("v2", rt["b1_zero"], rt["prof"], rt["groups"], rt["cuts"])
    nc = _CACHE.get(key)
    if nc is None:
        nc = _CACHE[key] = build_module(rt["prof"], rt["groups"], rt["cuts"],
                                        rt["b1_zero"])
    res = run_bass_kernel_spmd(nc, rt["maps"], core_ids=list(range(N_CORES)))
    Y_cores = [res.results[c]["Y"] for c in range(N_CORES)]
    return _combine(Y_cores, rt)


# ------------------------------------------------- host: fold, route, pack

def _pe_table():
    d_half = D // 2
    x = np.arange(RES, dtype=np.float64)[:, None]
    j = np.arange(d_half, dtype=np.float64)[None, :]
    pe = np.zeros((RES, d_half), np.float64)
    pe[:, 0::2] = np.sin(x / 50.0 ** (2.0 * j[:, 0::2] / d_half))
    pe[:, 1::2] = np.cos(x / 50.0 ** (2.0 * j[:, 1::2] / d_half))
    return pe


def _pe_idx(x, log10):
    x = x.astype(np.float32)
    if log10:
        x = np.float32(0.0025) * np.log2(x) ** 2
    x = np.maximum(x, np.float32(1.0 / RES))
    return np.clip(np.round(x * RES).astype(np.int64) - 1, 0, RES - 1)


def _split_pieces(counts, w0):
    """Even-split each expert's count into ceil(c/w0) pieces."""
    pieces = []
    for e, c in enumerate(counts):
        c = int(c)
        if c == 0:
            continue
        m = -(-c // w0)
        base, rem = divmod(c, m)
        lo = 0
        for i in range(m):
            n = base + (1 if i < rem else 0)
            pieces.append((e, lo, n))
            lo += n
    pieces.sort(key=lambda p: -p[2])
    return pieces


def _profile(pieces):
    P = -(-len(pieces) // N_CORES)
    prof = [-(-pieces[k * N_CORES][2] // 8) * 8 for k in range(P)]
    return P, prof


def _plan_groups(prof, cap=512):
    """Rank groups for batched gelu: first group is rank 0 alone (fast
    pipeline start); later groups pack up to 3 ranks while the group slot
    width fits the PSUM bank budget (cap)."""
    P = len(prof)
    gs = [(0, 1)]
    a = 1
    while a < P:
        b = a + 1
        width = prof[a]
        while b < P and b - a < 3 and width + prof[b] <= cap:
            width += prof[b]
            b += 1
        gs.append((a, b))
        a = b
    return tuple(gs)


def _route(inputs):
    src = np.asarray(inputs["src"]).astype(np.int64)
    frac = np.asarray(inputs["frac"], np.float32)
    f64 = lambda k: np.asarray(inputs[k], np.float64)
    cbfv, W_m2v, b_m2v = f64("cbfv"), f64("W_m2v"), f64("b_m2v")
    projW, projb = f64("projW"), f64("projb")
    routerW = f64("routerW")
    keys = f64("expert_keys")

    emb_sc = 2.0 ** f64("emb_scaler")[0]
    pe_sc = 2.0 ** (1.0 - f64("pos_scaler")[0]) ** 2
    ple_sc = 2.0 ** (1.0 - f64("pos_scaler_log")[0]) ** 2

    # folded per-view tables: h (proj) and r (proj @ router) per table row
    A0 = ((cbfv @ W_m2v + b_m2v) * emb_sc) @ projW[0] + projb[0]
    R0 = A0 @ routerW[0]
    pe_tab = _pe_table()
    H1 = (pe_tab * pe_sc) @ projW[1][:D // 2] + projb[1]
    R1 = H1 @ routerW[1]
    H2 = (pe_tab * ple_sc) @ projW[2][D // 2:] + projb[2]
    R2 = H2 @ routerW[2]

    sflat = src.reshape(-1)
    i1 = _pe_idx(frac, False).reshape(-1)
    i2 = _pe_idx(frac, True).reshape(-1)

    # dedupe: units = unique (view, table-row) pairs
    u0, inv0 = np.unique(sflat, return_inverse=True)
    u1, inv1 = np.unique(i1, return_inverse=True)
    u2, inv2 = np.unique(i2, return_inverse=True)
    n0, n1 = len(u0), len(u1)
    U = n0 + n1 + len(u2)
    h = np.concatenate([A0[u0], H1[u1], H2[u2]]).astype(np.float32)  # [U,64]
    r = np.concatenate([R0[u0], R1[u1], R2[u2]])                     # f64

    dist = np.sqrt(np.maximum(
        (r ** 2).sum(-1)[:, None]
        - 2.0 * (r @ keys.T)
        + (keys ** 2).sum(1)[None, :], 0.0))                         # [U,E]

    topi = np.argpartition(dist, K - 1, axis=1)[:, :K]               # [U,K]
    topd = np.take_along_axis(dist, topi, axis=1)
    g = np.exp(-(topd - topd.min(axis=1, keepdims=True)))
    g = (g / g.sum(axis=1, keepdims=True)).astype(np.float32)        # [U,K]

    b1 = np.asarray(inputs["b1"], np.float32)
    b1_zero = not b1.any()
    b2 = np.asarray(inputs["b2"], np.float32)
    b2c = np.einsum("uk,ukd->ud", g.astype(np.float64),
                    b2[topi].astype(np.float64)).astype(np.float32)  # [U,64]

    # expert -> assignment lists (unit id sorted by expert)
    flat_e = topi.reshape(-1)
    order = np.argsort(flat_e, kind="stable")
    counts = np.bincount(flat_e, minlength=E)
    un = np.repeat(np.arange(U), K)[order]
    offs = np.zeros(E + 1, np.int64)
    np.cumsum(counts, out=offs[1:])

    # pick the even-split width minimizing modeled per-core DMA-time cost
    best = None
    for w0 in (128, 160, 192, 224, 256, 320, 384, 448):
        P_, prof_ = _profile(_split_pieces(counts, w0))
        cost = 178.0 * P_ + 1.7 * sum(prof_)
        if best is None or cost < best[0]:
            best = (cost, w0)
    pieces = _split_pieces(counts, best[1])
    P, prof = _profile(pieces)
    prof = tuple(prof)
    off = np.zeros(P + 1, np.int64)
    np.cumsum(prof, out=off[1:])
    S = int(off[P])

    # rank-LPT: block k of 8 pieces -> rank k, largest to least-loaded core
    core_p = [[None] * P for _ in range(N_CORES)]
    load = np.zeros(N_CORES, np.int64)
    for k in range(P):
        blk = pieces[k * N_CORES:(k + 1) * N_CORES]
        for c, p in zip(np.argsort(load, kind="stable"), blk):
            core_p[int(c)][k] = p
            load[int(c)] += p[2]

    W1 = np.asarray(inputs["W1"], np.float32)
    W2 = np.asarray(inputs["W2"], np.float32)

    idx = np.zeros((U, K), np.int64)
    gats = np.zeros((U, K), np.float32)
    nxt = np.zeros(U, np.int64)
    # XW tensor layout (64 partitions): [W1(ranks<wa) | X(ranks<xcr) |
    # W1(ranks>=wa) | X(ranks>=xcr)] so chunk 1 = one contiguous DMA
    # carrying everything the first groups need
    xcr, wa = min(3, P), min(3, P)
    xc = int(off[xcr])
    seg1 = 256 * wa + xc                       # end of chunk 1
    xwtot = 256 * P + S

    def w1col(k):
        return 256 * k if k < wa else seg1 + 256 * (k - wa)

    def xcol(k):
        o = int(off[k])
        return 256 * wa + o if k < xcr else seg1 + 256 * (P - wa) + (o - xc)

    maps = []
    for c in range(N_CORES):
        XW = np.zeros((64, xwtot), np.float16)
        W2p = np.zeros((128, P * 128), np.float16)
        B1p = np.zeros((128, 2 * P), np.float32)
        for k in range(P):
            pc = core_p[c][k]
            if pc is None:
                continue
            e, lo, n = pc
            XW[:, w1col(k):w1col(k) + 256] = W1[e]
            W2p[:, 128 * k:128 * k + 64] = W2[e][0:128, :]
            W2p[:, 128 * k + 64:128 * k + 128] = W2[e][128:256, :]
            B1p[:, 2 * k] = b1[e, 0:128]
            B1p[:, 2 * k + 1] = b1[e, 128:256]
            sl = offs[e] + lo
            uu = un[sl:sl + n]
            XW[:, xcol(k):xcol(k) + n] = h[uu].T
            rr = nxt[uu]
            idx[uu, rr] = c * S + off[k] + np.arange(n)
            gats[uu, rr] = g.reshape(-1)[order[sl:sl + n]]
            nxt[uu] = rr + 1
        m = {"XW": XW, "W2b": W2p}
        if not b1_zero:
            m["B1"] = B1p
        maps.append(m)
    assert (nxt == K).all(), "every unit must get exactly K experts"

    groups = _plan_groups(prof)
    cuts = (xcr, wa, min(5, P), max(1, min(2, len(groups) - 2))
            if len(groups) > 1 else 0)

    hmask = ((frac * frac[:, :1]) != 0).astype(np.float32)
    return {"maps": maps, "idx": idx, "gats": gats, "b2c": b2c,
            "inv": (inv0, inv1, inv2), "nsz": (n0, n1),
            "hmask": hmask, "prof": prof, "groups": groups, "cuts": cuts,
            "b1_zero": b1_zero}


# ------------------------------------------------------------ device phase

def _segments(base, n, bank_cols):
    """Split [base, base+n) at bank_cols boundaries (psum tiles are
    bank-aligned; a matmul output must not cross a 2KB bank)."""
    segs = []
    o = 0
    while o < n:
        seg = min(n - o, bank_cols - ((base + o) % bank_cols))
        segs.append((o, seg))
        o += seg
    return segs


def build_module(prof, groups, cuts, b1_zero=True):
    P = len(prof)
    off = [0]
    for w in prof:
        off.append(off[-1] + w)
    S = off[P]
    xcr, wa, w2a, nout1 = cuts
    xc = off[xcr]
    seg1 = 256 * wa + xc
    xwtot = 256 * P + S
    gwmax = max(off[b] - off[a] for a, b in groups)

    def w1col(k):
        return 256 * k if k < wa else seg1 + 256 * (k - wa)

    def xcol(k):
        o = off[k]
        return 256 * wa + o if k < xcr else seg1 + 256 * (P - wa) + (o - xc)

    nc = bacc.Bacc("TRN2", target_bir_lowering=False, debug=False,
                   num_devices=N_CORES)
    xwt = nc.dram_tensor("XW", [64, xwtot], F16, kind="ExternalInput").ap()
    w2t = nc.dram_tensor("W2b", [128, P * 128], F16,
                         kind="ExternalInput").ap()
    b1t = None
    if not b1_zero:
        b1t = nc.dram_tensor("B1", [128, 2 * P], F32,
                             kind="ExternalInput").ap()
    yb = nc.dram_tensor("Y", [64, S], F16, kind="ExternalOutput").ap()

    with tile.TileContext(nc) as tc:
        with contextlib.ExitStack() as ctx:
            wp = ctx.enter_context(tc.tile_pool(name="wp", bufs=1))
            y1p = ctx.enter_context(tc.tile_pool(name="y1p", bufs=3))
            ps1p = ctx.enter_context(tc.tile_pool(name="ps1", bufs=2,
                                                  space="PSUM"))
            ps2p = ctx.enter_context(tc.tile_pool(name="ps2", bufs=3,
                                                  space="PSUM"))
            psdp = ctx.enter_context(tc.tile_pool(name="psd", bufs=1,
                                                  space="PSUM"))

            xw = wp.tile([64, xwtot], F16, tag="xw")
            w2sb = wp.tile([128, P * 128], F16, tag="w2sb")
            yo = wp.tile([64, S], F16, tag="yo")
            wrm = wp.tile([64, 16], F16, tag="wrm")
            b1sb = None
            if not b1_zero:
                b1sb = wp.tile([128, 2 * P], F32, tag="b1sb")

            # PE warm-up: the tensor engine clock ramp is anchored at the
            # first PE instruction; a cheap early matmul block pins the
            # anchor near t=0 so the real (DMA-gated) matmuls run at full
            # rate once the ~3us ramp has elapsed
            nc.vector.memset(wrm[:], 0.0)
            psd = psdp.tile([16, 16], F32, tag="psd")
            for _ in range(24):
                nc.tensor.matmul(psd[:], wrm[:], wrm[:],
                                 start=True, stop=True)

            # input stream: chunk 1 = W1+X for the first ranks in ONE
            # contiguous DMA (SP/HWDGE), chunk 2 = the rest; W2 streams
            # from the Pool SWDGE path (keeps HWDGE free for outputs)
            nc.sync.dma_start(xw[:, 0:seg1], xwt[:, 0:seg1])
            nc.gpsimd.dma_start(w2sb[:, 0:w2a * 128], w2t[:, 0:w2a * 128])
            if seg1 < xwtot:
                nc.sync.dma_start(xw[:, seg1:xwtot], xwt[:, seg1:xwtot])
            if P > w2a:
                nc.gpsimd.dma_start(w2sb[:, w2a * 128:P * 128],
                                    w2t[:, w2a * 128:P * 128])
            if b1sb is not None:
                nc.sync.dma_start(b1sb[:], b1t[:])

            def emit_y1(ga, gb):
                ps1 = ps1p.tile([128, 2 * gwmax], F32, tag="ps1")
                lo = 0
                for k in range(ga, gb):
                    w = prof[k]
                    for half in range(2):
                        l1 = xw[:, w1col(k) + 128 * half:
                                w1col(k) + 128 * half + 128]
                        for o, seg in _segments(lo, w, 512):
                            nc.tensor.matmul(
                                ps1[:, lo + o:lo + o + seg], l1,
                                xw[:, xcol(k) + o:xcol(k) + o + seg],
                                start=True, stop=True)
                        lo += w
                return ps1

            def emit_gelu(ps1, ga, gb):
                gw = off[gb] - off[ga]
                y1g = y1p.tile([128, 2 * gwmax], F16, tag="y1g")
                if b1_zero:
                    nc.scalar.activation(y1g[:, 0:2 * gw], ps1[:, 0:2 * gw],
                                         AF.Gelu)
                else:
                    lo = 0
                    for k in range(ga, gb):
                        w = prof[k]
                        for half in range(2):
                            nc.scalar.activation(
                                y1g[:, lo:lo + w], ps1[:, lo:lo + w],
                                AF.Gelu,
                                bias=b1sb[:, 2 * k + half:2 * k + half + 1])
                            lo += w
                return y1g


            def emit_y2(y1g, ga, gb, gi):
                last = gi == len(groups) - 1
                ps2 = ps2p.tile([64, gwmax], F32, tag="ps2")
                lo = 0
                for k in range(ga, gb):
                    w = prof[k]
                    l20 = w2sb[:, 128 * k:128 * k + 64]
                    l21 = w2sb[:, 128 * k + 64:128 * k + 128]
                    po = off[k] - off[ga]
                    for o, seg in _segments(po, w, 512):
                        nc.tensor.matmul(ps2[:, po + o:po + o + seg], l20,
                                         y1g[:, lo + o:lo + o + seg],
                                         start=True, stop=False)
                        nc.tensor.matmul(ps2[:, po + o:po + o + seg], l21,
                                         y1g[:, lo + w + o:lo + w + o + seg],
                                         start=False, stop=True)
                    lo += 2 * w
                gw = off[gb] - off[ga]
                obnd = off[groups[nout1 - 1][1]] if nout1 > 0 else 0
                if last:
                    nc.scalar.copy(yo[:, off[ga]:off[gb]], ps2[:, 0:gw])
                    nc.sync.dma_start(yb[:, obnd:S], yo[:, obnd:S])
                else:
                    nc.vector.tensor_copy(yo[:, off[ga]:off[gb]],
                                          ps2[:, 0:gw])
                    if gi == nout1 - 1 and nout1 > 0:
                        nc.sync.dma_start(yb[:, 0:obnd], yo[:, 0:obnd])

            # depth-2 software pipeline: y2 of group g is emitted after
            # y1 of group g+2, so the in-order PE queue never blocks the
            # next group's y1 behind a gelu wait
            pend = []
            for gi, (ga, gb) in enumerate(groups):
                ps1 = emit_y1(ga, gb)
                if len(pend) >= 2:
                    emit_y2(*pend.pop(0))
                y1g = emit_gelu(ps1, ga, gb)
                pend.append((y1g, ga, gb, gi))
            while pend:
                emit_y2(*pend.pop(0))
    nc.compile()
    return nc


# ------------------------------------------------------------ host combine

def _combine(Y_cores, rt):
    Yall = np.concatenate([np.asarray(y, np.float32).T for y in Y_cores])
    idx, gats = rt["idx"], rt["gats"]
    acc = rt["b2c"].copy()
    for r in range(K):
        acc += gats[:, r][:, None] * Yall[idx[:, r]]
    inv0, inv1, inv2 = rt["inv"]
    n0, n1 = rt["nsz"]
    out = acc[inv0] + acc[n0 + inv1] + acc[n0 + n1 + inv2]
    out = out.reshape(B, T, D) * rt["hmask"][:, :, None]
    return out.astype(np.float32)
